# revision 6
# baseline (speedup 1.0000x reference)
"""BayesianLinear (y = x @ (mu + softplus(rho) * eps).T + bias) on 8 TRN2 cores.

Column-parallel sharding: each core owns OUT_F/8 = 512 output features.

Host-side prep is pure layout/precision staging (no reference math):
  - x is cast to bf16 and pre-tiled into the SBUF layout the TensorEngine
    needs for its stationary operand: x_t[bt, pi, po, bi] = x[bt*128+bi,
    po*128+pi]. (An fp8-e4m3 DoubleRow variant of the trailing K-blocks
    was measured: walrus/TRN2 ran the DoubleRow matmuls at 1 elem/cycle
    — no ALU win — AND the presence of fp8 matmuls downclocked the PE
    2.4->2.0 GHz for the whole run, a 26us net loss. All-bf16 it is.)
  - weight_mu/rho/eps shards are transposed to [in_f, o_sh], tiled per
    128-row K-block, and PACKED into one bf16-typed tensor per K-block
    pair (mu bf16 | eps bf16 | rho fp16-bits) so W^T construction costs
    a single DMA per pair. rho ships as fp16 because softplus amplifies
    its quantization ~3x.

Device per core:
  1. Bias row (tiny DMAs first on the sync queue) and 16 packed param
     DMAs interleaved two-at-a-time between eight x sub-chunk loads of
     the 8-tile group, all on the sync HWDGE queue in hand-picked order:
     the first x sub-chunk leads so the k=0 matmuls aren't gated on the
     whole weight stream, and W^T pair j always lands just ahead of its
     consumption (the GPSIMD SWDGE queue the packs used to ride is ~9us
     slower to first byte, which idled the PE after warmup).
  2. softplus(rho) = Ln(1 + Exp(rho)) on ACT, mul/add on DVE writing
     bf16 into the resident W^T tile [128, 32, 512].
  3. PE program order: 20 warmup K=1 matmuls (HAM clock ramp + cover of
     the construction latency), bias broadcast (ones.T @ bias_bf) and
     its eviction (frees the 8th PSUM bank), then an 8-tile PSUM-bank
     group consuming K-blocks k-interleaved as construction produces
     them, then 56 streaming tiles one PSUM bank each: 32 accumulating
     bf16 matmuls, DVE eviction fused with the bias add, DMA out.
  4. The NEFF declares only the queues it uses (sync HWDGE 16 phys,
     SWDGE 2 phys) — the NRT start/end barrier expansions scan every
     physical queue, ~115ns each per engine, so the stock 50-queue
     layout burned ~6us at each end.
"""

import numpy as np
import ml_dtypes

import concourse.bacc as bacc
import concourse.mybir as mybir
import concourse.tile as tile
from concourse.bass_utils import run_bass_kernel_spmd

BATCH = 8192
IN_F = 4096
OUT_F = 4096
N_CORES = 8
P = 128
KF8 = 0  # fp8 DoubleRow K-blocks: disabled (downclocks the PE, no ALU win)

_NC_CACHE = {}


def build_nc(batch=BATCH, in_f=IN_F, o_sh=OUT_F // N_CORES, kf8=KF8):
    KB = in_f // P  # K-blocks of 128 along the contraction dim
    BT = batch // P  # 128-row output tiles
    K2 = 2 if KB % 2 == 0 else 1  # K-blocks per construction step
    NPAIR = KB // K2
    kbf = KB - kf8  # leading bf16 K-blocks
    assert kf8 % K2 == 0 and kbf % K2 == 0

    nc = bacc.Bacc(
        "TRN2",
        target_bir_lowering=False,
        debug=False,
        enable_asserts=False,
        num_devices=N_CORES,
    )
    bf16 = mybir.dt.bfloat16
    f16 = mybir.dt.float16
    f8 = mybir.dt.float8e4
    f32 = mybir.dt.float32

    # Drop the unused Activation HWDGE queue and shrink the SWDGE queue
    # (nothing rides it now): the NRT barrier expansion scans every
    # physical queue at both ends of the NEFF.
    nc.m.queues = [
        q if q.name != "qPoolDynamic" else mybir.DMAQueue(
            type=q.type, name=q.name, blocks=[], engine=q.engine,
            location_alt=q.location_alt, num_queues=2,
            num_semaphores=0, semaphores=[],
        )
        for q in nc.m.queues
        if q.name != "qScalarDynamicHW"
    ]

    xb = nc.declare_dram_parameter("x_bf", [BT, P, kbf, P], bf16, isOutput=False)
    x8 = (
        nc.declare_dram_parameter("x_f8", [BT, P, kf8, P], f8, isOutput=False)
        if kf8
        else None
    )
    wpk = nc.declare_dram_parameter(
        "wpk_t", [NPAIR, P, K2, 3 * o_sh], bf16, isOutput=False
    )
    bmu = nc.declare_dram_parameter("bias_mu", [1, o_sh], f32, isOutput=False)
    brho = nc.declare_dram_parameter("bias_rho", [1, o_sh], f32, isOutput=False)
    beps = nc.declare_dram_parameter("bias_eps", [1, o_sh], f32, isOutput=False)
    y = nc.declare_dram_parameter("y", [batch, o_sh], f32, isOutput=True)

    act_exp = mybir.ActivationFunctionType.Exp
    act_ln = mybir.ActivationFunctionType.Ln

    GROUP = 8
    N_WARM = 20

    with tile.TileContext(nc) as tc:
        with (
            tc.tile_pool(name="const", bufs=1) as const,
            tc.tile_pool(name="wcons", bufs=3) as wcons,
            tc.tile_pool(name="xin", bufs=13) as xin,
            tc.tile_pool(name="yout", bufs=4) as yout,
            tc.tile_pool(name="psum", bufs=8, space="PSUM") as psum_pool,
        ):
            bias_sb = const.tile([P, o_sh], f32, tag="bias_sb")
            bias_bf = const.tile([1, o_sh], bf16, tag="bias_bf")
            ones = const.tile([1, P], bf16, tag="ones")
            nc.vector.memset(ones[:], 1.0)
            wones = const.tile([1, o_sh], bf16, tag="wones")
            nc.vector.memset(wones[:], 1.0)

            # Bias inputs ride the sync queue ahead of everything (6 KiB).
            b_mu = const.tile([1, o_sh], f32, tag="b_mu")
            b_rho = const.tile([1, o_sh], f32, tag="b_rho")
            b_eps = const.tile([1, o_sh], f32, tag="b_eps")
            nc.sync.dma_start(out=b_mu[:], in_=bmu[:])
            nc.sync.dma_start(out=b_rho[:], in_=brho[:])
            nc.sync.dma_start(out=b_eps[:], in_=beps[:])
            b_sp = const.tile([1, o_sh], f32, tag="b_sp")
            nc.scalar.activation(b_sp[:], b_rho[:], act_exp)
            nc.scalar.activation(b_sp[:], b_sp[:], act_ln, bias=1.0)
            nc.vector.tensor_mul(out=b_sp[:], in0=b_sp[:], in1=b_eps[:])
            nc.vector.tensor_add(out=bias_bf[:], in0=b_sp[:], in1=b_mu[:])

            # ---- sync-queue DMA program: wpk pairs interleaved with the
            # group's x chunk loads so delivery tracks consumption order.
            # Each pair's construction ops (ACT softplus, DVE mul/add) are
            # emitted right after its DMA so the 3-deep pk ring's reuse
            # dependencies are in place before the ring wraps. The DVE
            # add's output dtype does the fp32->e4m3 rounding for the
            # fp8 blocks.
            WT_bf = const.tile([P, kbf, o_sh], bf16, tag="WT_bf")
            WT_f8 = const.tile([P, kf8, o_sh], f8, tag="WT_f8") if kf8 else None
            xbs = []
            x8s = []
            for bt in range(GROUP):
                xbs.append(xin.tile([P, kbf, P], bf16, tag="xT", name=f"xTb_g{bt}"))
                if kf8:
                    x8s.append(
                        xin.tile([P, kf8, P], f8, tag="x8", name=f"xT8_g{bt}")
                    )

            def emit_pair(j):
                pk = wcons.tile([P, K2, 3 * o_sh], bf16, tag="pk")
                nc.sync.dma_start(out=pk[:], in_=wpk[j])
                mu_t = pk[:, :, 0:o_sh]
                eps_t = pk[:, :, o_sh : 2 * o_sh]
                rho_t = pk[:, :, 2 * o_sh : 3 * o_sh].bitcast(f16)
                sp_t = wcons.tile([P, K2, o_sh], f32, tag="sp")
                nc.scalar.activation(sp_t[:], rho_t[:], act_exp)
                nc.scalar.activation(sp_t[:], sp_t[:], act_ln, bias=1.0)
                nc.vector.tensor_mul(out=sp_t[:], in0=sp_t[:], in1=eps_t[:])
                if j * K2 < kbf:
                    out_sl = WT_bf[:, j * K2 : (j + 1) * K2, :]
                else:
                    jf = j * K2 - kbf
                    out_sl = WT_f8[:, jf : jf + K2, :]
                nc.vector.tensor_add(out=out_sl, in0=sp_t[:], in1=mu_t[:])

            # x sub-chunks of ~4 K-blocks lead each pk pair so neither
            # stream starves the other; delivery order == queue order.
            NCH = 8
            bounds = [round(kbf * c / NCH) for c in range(NCH + 1)]
            order = []
            for c in range(NCH):
                order.append(("xc", c))
                order.append(("pk", 2 * c))
                order.append(("pk", 2 * c + 1))
            order += [("pk", j) for j in range(2 * NCH, NPAIR)]
            if kf8:
                order.append(("x8", 0))
            for kind, idx in order:
                if kind == "pk":
                    emit_pair(idx)
                elif kind == "xc":
                    ks = slice(bounds[idx], bounds[idx + 1])
                    for i in range(GROUP):
                        nc.sync.dma_start(out=xbs[i][:, ks, :], in_=xb[i, :, ks, :])
                else:
                    for i in range(GROUP):
                        nc.sync.dma_start(out=x8s[i][:], in_=x8[i])

            # ---- PE program: warmup (HAM ramp, covers construction
            # latency), bias broadcast, then the matmul stream.
            warm_ps = psum_pool.tile([P, o_sh], f32, tag="ps", name="warm_ps")
            for w in range(N_WARM):
                nc.tensor.matmul(warm_ps[:], lhsT=ones[:], rhs=wones[:])
            bias_ps = psum_pool.tile([P, o_sh], f32, tag="ps", name="bias_ps")
            nc.tensor.matmul(bias_ps[:], lhsT=ones[:], rhs=bias_bf[:])
            nc.vector.tensor_copy(out=bias_sb[:], in_=bias_ps[:])

            def emit_tile_mms(ps, xbf_t, xf8_t):
                for k in range(kbf):
                    nc.tensor.matmul(
                        ps[:],
                        lhsT=xbf_t[:, k, :],
                        rhs=WT_bf[:, k, :],
                        start=(k == 0),
                        stop=(k == kbf - 1 and not kf8),
                    )
                for j in range(kf8 // 2):
                    nc.tensor.matmul(
                        ps[:],
                        lhsT=xf8_t[:, 2 * j : 2 * j + 2, :],
                        rhs=WT_f8[:, 2 * j : 2 * j + 2, :],
                        start=False,
                        stop=(j == kf8 // 2 - 1),
                        perf_mode=mybir.MatmulPerfMode.DoubleRow,
                    )

            def body_tail(ps, bt):
                y_sb = yout.tile([P, o_sh], f32, tag="y_sb")
                nc.vector.tensor_add(out=y_sb[:], in0=ps[:], in1=bias_sb[:])
                nc.sync.dma_start(out=y[bt * P : (bt + 1) * P, :], in_=y_sb[:])

            # group: k-interleaved across the 8 PSUM banks so the PE
            # consumes W^T pairs no faster than construction makes them.
            pss = [
                psum_pool.tile([P, o_sh], f32, tag="ps", name=f"ps_g{bt}")
                for bt in range(GROUP)
            ]
            for k in range(kbf):
                for i in range(GROUP):
                    nc.tensor.matmul(
                        pss[i][:],
                        lhsT=xbs[i][:, k, :],
                        rhs=WT_bf[:, k, :],
                        start=(k == 0),
                        stop=(k == kbf - 1 and not kf8),
                    )
            for j in range(kf8 // 2):
                for i in range(GROUP):
                    nc.tensor.matmul(
                        pss[i][:],
                        lhsT=x8s[i][:, 2 * j : 2 * j + 2, :],
                        rhs=WT_f8[:, 2 * j : 2 * j + 2, :],
                        start=False,
                        stop=(j == kf8 // 2 - 1),
                        perf_mode=mybir.MatmulPerfMode.DoubleRow,
                    )
            for i in range(GROUP):
                body_tail(pss[i], i)

            # ---- remaining tiles stream one PSUM bank each
            for bt in range(GROUP, BT):
                xbf_t = xin.tile([P, kbf, P], bf16, tag="xT")
                nc.sync.dma_start(out=xbf_t[:], in_=xb[bt])
                xf8_t = None
                if kf8:
                    xf8_t = xin.tile([P, kf8, P], f8, tag="x8")
                    nc.sync.dma_start(out=xf8_t[:], in_=x8[bt])
                ps = psum_pool.tile([P, o_sh], f32, tag="ps")
                emit_tile_mms(ps, xbf_t, xf8_t)
                body_tail(ps, bt)

    # Skip bacc's pre-placed InstLoadActFuncSet: on large graphs walrus's
    # parallel-pass fork can separate the hoisted load from its activations
    # ("No Act func set exist for this instruction"); walrus's own lower_act
    # placement handles forked subgraphs correctly.
    nc.insert_act_table_loads = lambda: None
    nc.compile()
    return nc


def _prep_x(x, kf8=KF8):
    """[batch, in_f] fp32 -> (bf16 tiled [BT, 128, KB-kf8, 128],
    e4m3 tiled [BT, 128, kf8, 128] or None) with x_t[bt, pi, po, bi] =
    x[bt*128 + bi, po*128 + pi]."""
    batch, in_f = x.shape
    kcut = in_f - kf8 * P
    xbf = x[:, :kcut].astype(ml_dtypes.bfloat16)
    xbf = xbf.reshape(batch // P, P, kcut // P, P)  # [bt, bi, po, pi]
    xbf = np.ascontiguousarray(xbf.transpose(0, 3, 2, 1))  # [bt, pi, po, bi]
    if not kf8:
        return xbf, None
    xf8 = x[:, kcut:].astype(ml_dtypes.float8_e4m3)
    xf8 = xf8.reshape(batch // P, P, kf8, P)
    xf8 = np.ascontiguousarray(xf8.transpose(0, 3, 2, 1))
    return xbf, xf8


def _tile_w(w, dtype):
    """[o_sh, in_f] -> tiled [KB, 128, o_sh] with w_t[k, pi, o] = w[o, k*128 + pi]."""
    o_sh, in_f = w.shape
    return np.ascontiguousarray(w.T.reshape(in_f // P, P, o_sh)).astype(dtype)


def _prep_wpk(wmu, wrho, weps):
    """Pack mu (bf16), eps (bf16), rho (fp16 bits viewed as bf16) into one
    bf16-typed [KB/K2, 128, K2, 3*o_sh] tensor — one DMA per K2 K-blocks."""
    mu = _tile_w(wmu, ml_dtypes.bfloat16)
    eps = _tile_w(weps, ml_dtypes.bfloat16)
    rho = _tile_w(wrho, np.float16).view(ml_dtypes.bfloat16)
    pk = np.concatenate([mu, eps, rho], axis=2)  # [KB, P, 3*o_sh]
    kb, p, f = pk.shape
    k2 = 2 if kb % 2 == 0 else 1
    pk = pk.reshape(kb // k2, k2, p, f).transpose(0, 2, 1, 3)
    return np.ascontiguousarray(pk)


def make_in_maps(x, weight_mu, weight_rho, bias_mu, bias_rho, weight_eps, bias_eps):
    o_sh = OUT_F // N_CORES
    x_bf, x_f8 = _prep_x(np.asarray(x, dtype=np.float32))
    wmu = np.asarray(weight_mu, dtype=np.float32)
    wrho = np.asarray(weight_rho, dtype=np.float32)
    weps = np.asarray(weight_eps, dtype=np.float32)
    bmu = np.asarray(bias_mu, dtype=np.float32).reshape(1, -1)
    brho = np.asarray(bias_rho, dtype=np.float32).reshape(1, -1)
    beps = np.asarray(bias_eps, dtype=np.float32).reshape(1, -1)

    in_maps = []
    for c in range(N_CORES):
        rs = slice(c * o_sh, (c + 1) * o_sh)
        im = {
                "x_bf": x_bf,
                "wpk_t": _prep_wpk(wmu[rs], wrho[rs], weps[rs]),
                "bias_mu": np.ascontiguousarray(bmu[:, rs]),
                "bias_rho": np.ascontiguousarray(brho[:, rs]),
                "bias_eps": np.ascontiguousarray(beps[:, rs]),
        }
        if x_f8 is not None:
            im["x_f8"] = x_f8
        in_maps.append(im)
    return in_maps


def kernel(x, weight_mu, weight_rho, bias_mu, bias_rho, weight_eps, bias_eps):
    o_sh = OUT_F // N_CORES
    key = (x.shape, o_sh)
    if key not in _NC_CACHE:
        _NC_CACHE[key] = build_nc(x.shape[0], x.shape[1], o_sh)
    nc = _NC_CACHE[key]

    in_maps = make_in_maps(
        x, weight_mu, weight_rho, bias_mu, bias_rho, weight_eps, bias_eps
    )
    res = run_bass_kernel_spmd(nc, in_maps, core_ids=list(range(N_CORES)))
    return np.concatenate([res.results[c]["y"] for c in range(N_CORES)], axis=1)


# revision 7
# speedup vs baseline: 1.0158x; 1.0158x over previous
"""BayesianLinear (y = x @ (mu + softplus(rho) * eps).T + bias) on 8 TRN2 cores.

Column-parallel sharding: each core owns OUT_F/8 = 512 output features.

Host-side prep is pure layout/precision staging (no reference math):
  - x is cast to bf16 and pre-tiled into the SBUF layout the TensorEngine
    needs for its stationary operand: x_t[bt, pi, po, bi] = x[bt*128+bi,
    po*128+pi]. (An fp8-e4m3 DoubleRow variant of the trailing K-blocks
    was measured: walrus/TRN2 ran the DoubleRow matmuls at 1 elem/cycle
    — no ALU win — AND the presence of fp8 matmuls downclocked the PE
    2.4->2.0 GHz for the whole run, a 26us net loss. All-bf16 it is.)
  - weight_mu/rho/eps shards are transposed to [in_f, o_sh], tiled per
    128-row K-block, and PACKED into one bf16-typed tensor per K-block
    pair (mu bf16 | eps bf16 | rho fp16-bits) so W^T construction costs
    a single DMA per pair. rho ships as fp16 because softplus amplifies
    its quantization ~3x.

Device per core:
  1. Bias row (tiny DMAs first on the sync queue) and 16 packed param
     DMAs interleaved with the 8-tile group's x chunk loads in
     hand-picked order (pk pairs lead; W^T pair j always lands just
     ahead of its consumption). Each DMA_DIRECT2D trigger costs ~0.7us
     of issuing-engine time, so the group's later x chunks are triggered
     from the scalar engine's HWDGE queue in the slack between softplus
     ops — one queue's trigger rate alone caps phase-1 x delivery below
     the PE's consumption rate. (The GPSIMD SWDGE queue the packs used
     to ride is ~9us slower to first byte, which idled the PE.)
  2. softplus(rho) = Ln(1 + Exp(rho)) on ACT, mul/add on DVE writing
     bf16 into the resident W^T tile [128, 32, 512].
  3. PE program order: 14 warmup K=1 matmuls (HAM clock ramp + cover of
     the construction latency), bias broadcast (ones.T @ bias_bf) and
     its eviction (frees the 8th PSUM bank), then an 8-tile PSUM-bank
     group consuming K-blocks k-interleaved as construction produces
     them, then 56 streaming tiles one PSUM bank each: 32 accumulating
     bf16 matmuls, DVE eviction fused with the bias add, DMA out.
  4. The NEFF declares only the queues it uses (sync HWDGE 16 phys,
     SWDGE 2 phys) — the NRT start/end barrier expansions scan every
     physical queue, ~115ns each per engine, so the stock 50-queue
     layout burned ~6us at each end.
"""

import numpy as np
import ml_dtypes

import concourse.bacc as bacc
import concourse.mybir as mybir
import concourse.tile as tile
from concourse.bass_utils import run_bass_kernel_spmd

BATCH = 8192
IN_F = 4096
OUT_F = 4096
N_CORES = 8
P = 128
KF8 = 0  # fp8 DoubleRow K-blocks: disabled (downclocks the PE, no ALU win)

_NC_CACHE = {}


def build_nc(batch=BATCH, in_f=IN_F, o_sh=OUT_F // N_CORES, kf8=KF8):
    KB = in_f // P  # K-blocks of 128 along the contraction dim
    BT = batch // P  # 128-row output tiles
    K2 = 2 if KB % 2 == 0 else 1  # K-blocks per construction step
    NPAIR = KB // K2
    kbf = KB - kf8  # leading bf16 K-blocks
    assert kf8 % K2 == 0 and kbf % K2 == 0

    nc = bacc.Bacc(
        "TRN2",
        target_bir_lowering=False,
        debug=False,
        enable_asserts=False,
        num_devices=N_CORES,
    )
    bf16 = mybir.dt.bfloat16
    f16 = mybir.dt.float16
    f8 = mybir.dt.float8e4
    f32 = mybir.dt.float32

    # Shrink the unused SWDGE queue (nothing rides it; bias moved to the
    # sync queue) — the NRT barrier expansion scans every physical queue
    # at both ends of the NEFF.
    nc.m.queues = [
        q if q.name != "qPoolDynamic" else mybir.DMAQueue(
            type=q.type, name=q.name, blocks=[], engine=q.engine,
            location_alt=q.location_alt, num_queues=2,
            num_semaphores=0, semaphores=[],
        )
        for q in nc.m.queues
    ]

    xb = nc.declare_dram_parameter("x_bf", [BT, P, kbf, P], bf16, isOutput=False)
    x8 = (
        nc.declare_dram_parameter("x_f8", [BT, P, kf8, P], f8, isOutput=False)
        if kf8
        else None
    )
    wpk = nc.declare_dram_parameter(
        "wpk_t", [NPAIR, P, K2, 3 * o_sh], bf16, isOutput=False
    )
    bmu = nc.declare_dram_parameter("bias_mu", [1, o_sh], f32, isOutput=False)
    brho = nc.declare_dram_parameter("bias_rho", [1, o_sh], f32, isOutput=False)
    beps = nc.declare_dram_parameter("bias_eps", [1, o_sh], f32, isOutput=False)
    y = nc.declare_dram_parameter("y", [batch, o_sh], f32, isOutput=True)

    act_exp = mybir.ActivationFunctionType.Exp
    act_ln = mybir.ActivationFunctionType.Ln

    GROUP = 8
    N_WARM = 14

    with tile.TileContext(nc) as tc:
        with (
            tc.tile_pool(name="const", bufs=1) as const,
            tc.tile_pool(name="wcons", bufs=3) as wcons,
            tc.tile_pool(name="xin", bufs=13) as xin,
            tc.tile_pool(name="yout", bufs=4) as yout,
            tc.tile_pool(name="psum", bufs=8, space="PSUM") as psum_pool,
        ):
            bias_sb = const.tile([P, o_sh], f32, tag="bias_sb")
            bias_bf = const.tile([1, o_sh], bf16, tag="bias_bf")
            ones = const.tile([1, P], bf16, tag="ones")
            nc.vector.memset(ones[:], 1.0)
            wones = const.tile([1, o_sh], bf16, tag="wones")
            nc.vector.memset(wones[:], 1.0)

            # Bias inputs ride the sync queue ahead of everything (6 KiB).
            b_mu = const.tile([1, o_sh], f32, tag="b_mu")
            b_rho = const.tile([1, o_sh], f32, tag="b_rho")
            b_eps = const.tile([1, o_sh], f32, tag="b_eps")
            nc.sync.dma_start(out=b_mu[:], in_=bmu[:])
            nc.sync.dma_start(out=b_rho[:], in_=brho[:])
            nc.sync.dma_start(out=b_eps[:], in_=beps[:])
            b_sp = const.tile([1, o_sh], f32, tag="b_sp")
            nc.scalar.activation(b_sp[:], b_rho[:], act_exp)
            nc.scalar.activation(b_sp[:], b_sp[:], act_ln, bias=1.0)
            nc.vector.tensor_mul(out=b_sp[:], in0=b_sp[:], in1=b_eps[:])
            nc.vector.tensor_add(out=bias_bf[:], in0=b_sp[:], in1=b_mu[:])

            # ---- sync-queue DMA program: wpk pairs interleaved with the
            # group's x chunk loads so delivery tracks consumption order.
            # Each pair's construction ops (ACT softplus, DVE mul/add) are
            # emitted right after its DMA so the 3-deep pk ring's reuse
            # dependencies are in place before the ring wraps. The DVE
            # add's output dtype does the fp32->e4m3 rounding for the
            # fp8 blocks.
            WT_bf = const.tile([P, kbf, o_sh], bf16, tag="WT_bf")
            WT_f8 = const.tile([P, kf8, o_sh], f8, tag="WT_f8") if kf8 else None
            xbs = []
            x8s = []
            for bt in range(GROUP):
                xbs.append(xin.tile([P, kbf, P], bf16, tag="xT", name=f"xTb_g{bt}"))
                if kf8:
                    x8s.append(
                        xin.tile([P, kf8, P], f8, tag="x8", name=f"xT8_g{bt}")
                    )

            def emit_pair(j):
                pk = wcons.tile([P, K2, 3 * o_sh], bf16, tag="pk")
                nc.sync.dma_start(out=pk[:], in_=wpk[j])
                mu_t = pk[:, :, 0:o_sh]
                eps_t = pk[:, :, o_sh : 2 * o_sh]
                rho_t = pk[:, :, 2 * o_sh : 3 * o_sh].bitcast(f16)
                sp_t = wcons.tile([P, K2, o_sh], f32, tag="sp")
                nc.scalar.activation(sp_t[:], rho_t[:], act_exp)
                nc.scalar.activation(sp_t[:], sp_t[:], act_ln, bias=1.0)
                nc.vector.tensor_mul(out=sp_t[:], in0=sp_t[:], in1=eps_t[:])
                if j * K2 < kbf:
                    out_sl = WT_bf[:, j * K2 : (j + 1) * K2, :]
                else:
                    jf = j * K2 - kbf
                    out_sl = WT_f8[:, jf : jf + K2, :]
                nc.vector.tensor_add(out=out_sl, in0=sp_t[:], in1=mu_t[:])

            # Interleave: pk pairs lead (longest latency chain), the first
            # x chunk rides the sync queue, later chunks are triggered by
            # the scalar engine's HWDGE queue — each DMA_DIRECT2D trigger
            # costs ~0.7us of issuing-engine time, so 32 group-chunk
            # triggers on one queue would cap x delivery below the PE's
            # consumption rate. Emission order == per-queue issue order.
            NCH = 4
            bounds = [round(kbf * c / NCH) for c in range(NCH + 1)]
            order = []
            order += [("pk", 0), ("pk", 1), ("xc", 0)]
            order += [("pk", 2), ("pk", 3), ("xc", 1)]
            order += [("pk", 4), ("pk", 5), ("xc", 2)]
            order += [("pk", 6), ("pk", 7), ("xc", 3)]
            order += [("pk", j) for j in range(8, NPAIR)]
            if kf8:
                order.append(("x8", 0))
            for kind, idx in order:
                if kind == "pk":
                    emit_pair(idx)
                elif kind == "xc":
                    ks = slice(bounds[idx], bounds[idx + 1])
                    eng = nc.sync if idx == 0 else nc.scalar
                    for i in range(GROUP):
                        eng.dma_start(out=xbs[i][:, ks, :], in_=xb[i, :, ks, :])
                else:
                    for i in range(GROUP):
                        nc.sync.dma_start(out=x8s[i][:], in_=x8[i])

            # ---- PE program: warmup (HAM ramp, covers construction
            # latency), bias broadcast, then the matmul stream.
            warm_ps = psum_pool.tile([P, o_sh], f32, tag="ps", name="warm_ps")
            for w in range(N_WARM):
                nc.tensor.matmul(warm_ps[:], lhsT=ones[:], rhs=wones[:])
            bias_ps = psum_pool.tile([P, o_sh], f32, tag="ps", name="bias_ps")
            nc.tensor.matmul(bias_ps[:], lhsT=ones[:], rhs=bias_bf[:])
            nc.vector.tensor_copy(out=bias_sb[:], in_=bias_ps[:])

            def emit_tile_mms(ps, xbf_t, xf8_t):
                for k in range(kbf):
                    nc.tensor.matmul(
                        ps[:],
                        lhsT=xbf_t[:, k, :],
                        rhs=WT_bf[:, k, :],
                        start=(k == 0),
                        stop=(k == kbf - 1 and not kf8),
                    )
                for j in range(kf8 // 2):
                    nc.tensor.matmul(
                        ps[:],
                        lhsT=xf8_t[:, 2 * j : 2 * j + 2, :],
                        rhs=WT_f8[:, 2 * j : 2 * j + 2, :],
                        start=False,
                        stop=(j == kf8 // 2 - 1),
                        perf_mode=mybir.MatmulPerfMode.DoubleRow,
                    )

            def body_tail(ps, bt):
                y_sb = yout.tile([P, o_sh], f32, tag="y_sb")
                nc.vector.tensor_add(out=y_sb[:], in0=ps[:], in1=bias_sb[:])
                nc.sync.dma_start(out=y[bt * P : (bt + 1) * P, :], in_=y_sb[:])

            # group: k-interleaved across the 8 PSUM banks so the PE
            # consumes W^T pairs no faster than construction makes them.
            pss = [
                psum_pool.tile([P, o_sh], f32, tag="ps", name=f"ps_g{bt}")
                for bt in range(GROUP)
            ]
            for k in range(kbf):
                for i in range(GROUP):
                    nc.tensor.matmul(
                        pss[i][:],
                        lhsT=xbs[i][:, k, :],
                        rhs=WT_bf[:, k, :],
                        start=(k == 0),
                        stop=(k == kbf - 1 and not kf8),
                    )
            for j in range(kf8 // 2):
                for i in range(GROUP):
                    nc.tensor.matmul(
                        pss[i][:],
                        lhsT=x8s[i][:, 2 * j : 2 * j + 2, :],
                        rhs=WT_f8[:, 2 * j : 2 * j + 2, :],
                        start=False,
                        stop=(j == kf8 // 2 - 1),
                        perf_mode=mybir.MatmulPerfMode.DoubleRow,
                    )
            for i in range(GROUP):
                body_tail(pss[i], i)

            # ---- remaining tiles stream one PSUM bank each
            for bt in range(GROUP, BT):
                xbf_t = xin.tile([P, kbf, P], bf16, tag="xT")
                nc.sync.dma_start(out=xbf_t[:], in_=xb[bt])
                xf8_t = None
                if kf8:
                    xf8_t = xin.tile([P, kf8, P], f8, tag="x8")
                    nc.sync.dma_start(out=xf8_t[:], in_=x8[bt])
                ps = psum_pool.tile([P, o_sh], f32, tag="ps")
                emit_tile_mms(ps, xbf_t, xf8_t)
                body_tail(ps, bt)

    # Skip bacc's pre-placed InstLoadActFuncSet: on large graphs walrus's
    # parallel-pass fork can separate the hoisted load from its activations
    # ("No Act func set exist for this instruction"); walrus's own lower_act
    # placement handles forked subgraphs correctly.
    nc.insert_act_table_loads = lambda: None
    nc.compile()
    return nc


def _prep_x(x, kf8=KF8):
    """[batch, in_f] fp32 -> (bf16 tiled [BT, 128, KB-kf8, 128],
    e4m3 tiled [BT, 128, kf8, 128] or None) with x_t[bt, pi, po, bi] =
    x[bt*128 + bi, po*128 + pi]."""
    batch, in_f = x.shape
    kcut = in_f - kf8 * P
    xbf = x[:, :kcut].astype(ml_dtypes.bfloat16)
    xbf = xbf.reshape(batch // P, P, kcut // P, P)  # [bt, bi, po, pi]
    xbf = np.ascontiguousarray(xbf.transpose(0, 3, 2, 1))  # [bt, pi, po, bi]
    if not kf8:
        return xbf, None
    xf8 = x[:, kcut:].astype(ml_dtypes.float8_e4m3)
    xf8 = xf8.reshape(batch // P, P, kf8, P)
    xf8 = np.ascontiguousarray(xf8.transpose(0, 3, 2, 1))
    return xbf, xf8


def _tile_w(w, dtype):
    """[o_sh, in_f] -> tiled [KB, 128, o_sh] with w_t[k, pi, o] = w[o, k*128 + pi]."""
    o_sh, in_f = w.shape
    return np.ascontiguousarray(w.T.reshape(in_f // P, P, o_sh)).astype(dtype)


def _prep_wpk(wmu, wrho, weps):
    """Pack mu (bf16), eps (bf16), rho (fp16 bits viewed as bf16) into one
    bf16-typed [KB/K2, 128, K2, 3*o_sh] tensor — one DMA per K2 K-blocks."""
    mu = _tile_w(wmu, ml_dtypes.bfloat16)
    eps = _tile_w(weps, ml_dtypes.bfloat16)
    rho = _tile_w(wrho, np.float16).view(ml_dtypes.bfloat16)
    pk = np.concatenate([mu, eps, rho], axis=2)  # [KB, P, 3*o_sh]
    kb, p, f = pk.shape
    k2 = 2 if kb % 2 == 0 else 1
    pk = pk.reshape(kb // k2, k2, p, f).transpose(0, 2, 1, 3)
    return np.ascontiguousarray(pk)


def make_in_maps(x, weight_mu, weight_rho, bias_mu, bias_rho, weight_eps, bias_eps):
    o_sh = OUT_F // N_CORES
    x_bf, x_f8 = _prep_x(np.asarray(x, dtype=np.float32))
    wmu = np.asarray(weight_mu, dtype=np.float32)
    wrho = np.asarray(weight_rho, dtype=np.float32)
    weps = np.asarray(weight_eps, dtype=np.float32)
    bmu = np.asarray(bias_mu, dtype=np.float32).reshape(1, -1)
    brho = np.asarray(bias_rho, dtype=np.float32).reshape(1, -1)
    beps = np.asarray(bias_eps, dtype=np.float32).reshape(1, -1)

    in_maps = []
    for c in range(N_CORES):
        rs = slice(c * o_sh, (c + 1) * o_sh)
        im = {
                "x_bf": x_bf,
                "wpk_t": _prep_wpk(wmu[rs], wrho[rs], weps[rs]),
                "bias_mu": np.ascontiguousarray(bmu[:, rs]),
                "bias_rho": np.ascontiguousarray(brho[:, rs]),
                "bias_eps": np.ascontiguousarray(beps[:, rs]),
        }
        if x_f8 is not None:
            im["x_f8"] = x_f8
        in_maps.append(im)
    return in_maps


def kernel(x, weight_mu, weight_rho, bias_mu, bias_rho, weight_eps, bias_eps):
    o_sh = OUT_F // N_CORES
    key = (x.shape, o_sh)
    if key not in _NC_CACHE:
        _NC_CACHE[key] = build_nc(x.shape[0], x.shape[1], o_sh)
    nc = _NC_CACHE[key]

    in_maps = make_in_maps(
        x, weight_mu, weight_rho, bias_mu, bias_rho, weight_eps, bias_eps
    )
    res = run_bass_kernel_spmd(nc, in_maps, core_ids=list(range(N_CORES)))
    return np.concatenate([res.results[c]["y"] for c in range(N_CORES)], axis=1)
